# revision 8
# baseline (speedup 1.0000x reference)
"""Trainium2 Bass kernel for nn_ConstituencyLBP (B=8, L=128, MAX_ITER=3).

Math reduction (validated against the jax reference to ~1e-5):

Within one batch element b, the LBP loop decomposes over the second span
index x into L independent "slabs".  Per slab x, only two things evolve:

  D[alpha, delta] = mp1 - mp0           (2-channel log-softmax difference)
  dq[alpha]       = q1 - q0

with the recurrence (S[alpha, delta] = s_pair[b, alpha, x, delta]):

  r   = dq[alpha] - D
  D'  = softplus(r + S) - softplus(r)
  agg[a]  = sum_k D'[k, a] - D'[a, a] - D'[x, a]
  dq' = s_span[b, a, x] + maskT[a, x] * agg[a]

and the output is out[b, i, j] = sigmoid(dq_{x=j}[i]).

This toolchain's ACT tables don't expose softplus, so the kernel works in
the exp domain: state W = exp(r), constant eS = exp(S) (precomputed once
in SBUF), and

  sp1 = Ln(W*eS + 1),  sp0 = Ln(W + 1),  D' = sp1 - sp0
  W'  = Exp(dq'[alpha] - D')

(empirically r <= ~51 and r+S <= ~48 for this problem's inputs, far below
f32 exp overflow at 88; Ln(x+1) loses nothing for x >= 0).

One core per batch element.  All 128 slabs of a core stay resident in SBUF
([128, 128, 128] f32 planes); the masked aggregation sum_k D'[k,a] *
(1 - delta(k,x)) is one [128,128]x[128,1] matmul per slab (lhsT = D'
plane, rhs = column x of V = 1 - I).  The diagonal D'[a,a] is tracked by
an identical per-column recurrence (sdiag[a,x] = s_pair[b,a,x,a]) rather
than being extracted from the plane.
"""

import numpy as np

import concourse.bacc as bacc
import concourse.tile as tile
from concourse import mybir
from concourse.bass_utils import run_bass_kernel_spmd

L = 128
N_CORES = 8
MAX_ITER = 3
G = 8                 # slabs per instruction group
NG = L // G           # groups
CLAMP = 25.0          # softplus(x) == x (to 1e-8) above this; keeps exp in table range
F32 = mybir.dt.float32
AF = mybir.ActivationFunctionType

_NC_CACHE = {}


def _bcast_col(col_ap, sl, g):
    # [128, L] column tile sliced to [128, g] then broadcast to [128, g, L]
    return col_ap[:, sl, None].to_broadcast((L, g, L))


def _softplus_cols(nc, out, in_, scr):
    # out = Ln(Exp(in_) + 1) on [128, L] column tiles
    nc.scalar.activation(scr, in_, AF.Exp)
    nc.scalar.activation(out, scr, AF.Ln, bias=1.0)


def _build_nc():
    nc = bacc.Bacc(None)
    sp_d = nc.dram_tensor("sp", [L, L, L], F32, kind="ExternalInput")
    sspan_d = nc.dram_tensor("sspan", [L, L], F32, kind="ExternalInput")
    maskt_d = nc.dram_tensor("maskt", [L, L], F32, kind="ExternalInput")
    sdiag_d = nc.dram_tensor("sdiag", [L, L], F32, kind="ExternalInput")
    vmat_d = nc.dram_tensor("vmat", [L, L], F32, kind="ExternalInput")
    out_d = nc.dram_tensor("out", [L, L], F32, kind="ExternalOutput")

    with tile.TileContext(nc) as tc:
        with (
            tc.tile_pool(name="big", bufs=1) as big,
            tc.tile_pool(name="cols", bufs=1) as cols,
            tc.tile_pool(name="scr", bufs=3) as scr,
            tc.tile_pool(name="colscr", bufs=2) as colscr,
            tc.tile_pool(name="dqp", bufs=2) as dqp,
            tc.tile_pool(name="ddp", bufs=2) as ddp,
            tc.tile_pool(name="psum", bufs=2, space="PSUM") as psum,
        ):
            es_all = big.tile([L, L, L], F32)    # exp(S)[alpha, x, delta]
            w_all = big.tile([L, L, L], F32)     # W / D' / F' plane per slab

            sspan_sb = cols.tile([L, L], F32)
            maskt_sb = cols.tile([L, L], F32)
            sdiag_sb = cols.tile([L, L], F32)
            vmat_sb = cols.tile([L, L], F32)
            nc.sync.dma_start(sspan_sb, sspan_d[:, :])
            nc.sync.dma_start(maskt_sb, maskt_d[:, :])
            nc.sync.dma_start(sdiag_sb, sdiag_d[:, :])
            nc.sync.dma_start(vmat_sb, vmat_d[:, :])
            for g in range(NG):
                sl = slice(g * G, (g + 1) * G)
                nc.sync.dma_start(es_all[:, sl, :], sp_d[:, sl, :])
                nc.scalar.activation(es_all[:, sl, :], es_all[:, sl, :], AF.Exp)

            # exp(dq0) and softplus(dq0) columns for the first iteration
            expdq0 = cols.tile([L, L], F32)
            sp0c = cols.tile([L, L], F32)
            nc.scalar.activation(expdq0, sspan_sb, AF.Exp)
            nc.scalar.activation(sp0c, expdq0, AF.Ln, bias=1.0)

            ddiag = ddp.tile([L, L], F32, tag="ddiag")
            nc.vector.memset(ddiag, 0.0)
            dq_cur = sspan_sb

            for it in range(MAX_ITER):
                # --- diagonal recurrence ([128, L] column ops) ---
                u0 = colscr.tile([L, L], F32, tag="u0")
                td = colscr.tile([L, L], F32, tag="td")
                cs = colscr.tile([L, L], F32, tag="cs")
                nc.vector.tensor_sub(u0, dq_cur, ddiag)
                # r <= ~51 here exceeds the ACT exp/ln table range; softplus
                # is exactly linear above 25 so the clamp is error-free
                nc.vector.tensor_scalar_min(u0, u0, CLAMP)
                nc.vector.tensor_add(td, u0, sdiag_sb)
                _softplus_cols(nc, u0, u0, cs)
                _softplus_cols(nc, td, td, cs)
                ddiag_new = ddp.tile([L, L], F32, tag="ddiag")
                nc.vector.tensor_sub(ddiag_new, td, u0)

                # --- plane recurrence + per-slab aggregation matmuls ---
                psum_agg = psum.tile([L, L], F32, tag="agg")
                for g in range(NG):
                    sl = slice(g * G, (g + 1) * G)
                    wg = w_all[:, sl, :]
                    esg = es_all[:, sl, :]
                    t1 = scr.tile([L, G, L], F32, tag="t1")
                    if it == 0:
                        # W0 = exp(dq0) broadcast; never materialized
                        nc.vector.tensor_mul(t1, esg, _bcast_col(expdq0, sl, G))
                        nc.scalar.activation(t1, t1, AF.Ln, bias=1.0)   # sp1
                        nc.vector.tensor_sub(wg, t1, _bcast_col(sp0c, sl, G))
                    else:
                        nc.vector.tensor_mul(t1, esg, wg)
                        nc.scalar.activation(t1, t1, AF.Ln, bias=1.0)   # sp1
                        nc.scalar.activation(wg, wg, AF.Ln, bias=1.0)   # sp0
                        nc.vector.tensor_sub(wg, t1, wg)
                    # wg now holds D' for these slabs
                    for x in range(g * G, (g + 1) * G):
                        nc.tensor.matmul(
                            psum_agg[:, x : x + 1],
                            w_all[:, x, :],
                            vmat_sb[:, x : x + 1],
                            start=True,
                            stop=True,
                        )

                # --- dq' assembly ---
                dq_new = dqp.tile([L, L], F32, tag="dq")
                nc.vector.tensor_sub(dq_new, psum_agg, ddiag_new)
                nc.vector.tensor_mul(dq_new, dq_new, maskt_sb)
                nc.vector.tensor_add(dq_new, dq_new, sspan_sb)

                # --- next state: W' = Exp(dq' - D') ---
                if it < MAX_ITER - 1:
                    for g in range(NG):
                        sl = slice(g * G, (g + 1) * G)
                        wg = w_all[:, sl, :]
                        nc.vector.tensor_sub(wg, _bcast_col(dq_new, sl, G), wg)
                        nc.gpsimd.tensor_scalar_min(wg, wg, CLAMP)
                        nc.scalar.activation(wg, wg, AF.Exp)

                ddiag = ddiag_new
                dq_cur = dq_new

            out_sb = cols.tile([L, L], F32)
            nc.scalar.activation(out_sb, dq_cur, AF.Sigmoid)
            nc.sync.dma_start(out_d[:, :], out_sb)

    return nc


def _get_nc():
    if "nc" not in _NC_CACHE:
        nc = _build_nc()
        if not nc.is_finalized():
            nc.finalize()
        _NC_CACHE["nc"] = nc
    return _NC_CACHE["nc"]


def _make_in_maps(s_span, s_pair, mask):
    ar = np.arange(L)
    vmat = (1.0 - np.eye(L)).astype(np.float32)
    in_maps = []
    for b in range(N_CORES):
        spb = np.ascontiguousarray(np.asarray(s_pair[b], np.float32))
        in_maps.append(
            {
                "sp": spb,
                "sspan": np.ascontiguousarray(np.asarray(s_span[b], np.float32)),
                "maskt": np.ascontiguousarray(np.asarray(mask[b]).T.astype(np.float32)),
                "sdiag": np.ascontiguousarray(spb[ar[:, None], ar[None, :], ar[:, None]]),
                "vmat": vmat,
            }
        )
    return in_maps


def kernel(s_span, s_pair, mask):
    nc = _get_nc()
    in_maps = _make_in_maps(s_span, s_pair, mask)
    res = run_bass_kernel_spmd(nc, in_maps, core_ids=list(range(N_CORES)))
    return np.stack([res.results[b]["out"] for b in range(N_CORES)])
